# revision 20
# baseline (speedup 1.0000x reference)
"""CostVolume2D Trainium2 kernel (v2: skewed-PSUM compact store).

out[b, d, h, w] = mean_c l[b,c,h,w] * r[b,c,h, w - (d - maxd)]  (zero padded)

Strategy (8 NeuronCores, shard H — no halo since shifts only touch W):
  * Per (b, h): disparity planes are diagonals of banded gram matrices
    G[w, w'] = sum_c l[c, w] r[c, w'] with |w - w'| <= 48.
  * v2 change vs v1: instead of computing full [128 x 224] gram blocks
    and storing all 224 columns per row (2.31x write amplification),
    each 128-row block is split into 128/WG row-groups; the matmul for
    group g streams only the needed column window [WG*g, WG*g+WG+96)
    and writes it to a PSUM window shared across groups. The diagonal
    band is then pre-aligned per group in PSUM, the eviction copies the
    full 128-partition tile at full lane utilization, and the store DMA
    absorbs the residual per-row skew (+1/row within a group) with a
    3-dim flat-stride access pattern. Write amplification drops to
    (WG+96)/97 and the quadrant matmuls (K=64, M=WG) run concurrently
    on disjoint PE array tiles (tile_position auto-derived from the
    lhsT/out base partitions).
  * 4 h-rows are packed per store DMA so descriptors are 8*(WG+96)
    bytes; loads use a host-side relayout so each load descriptor is
    4 h-pairs x 1120 elems (8960 B) contiguous.
  * Host pre-divides l by C (exact, power of two) and pre-pads r along
    W so no on-device scaling / memset / edge handling.
"""

import sys

try:
    import concourse  # noqa: F401
except ImportError:
    sys.path.insert(0, "/opt/trn_rl_repo")

import numpy as np

from concourse import bass, mybir
from concourse import tile
from concourse.ap import AP
from concourse.bass_utils import run_bass_kernel_spmd

F32 = mybir.dt.float32
F16 = mybir.dt.float16

# Problem dims (hardcoded per spec)
B, C, H, W = 4, 64, 256, 512
MAXD = 48
D = 2 * MAXD + 1          # 97 disparity planes
NCORES = 8
HS = H // NCORES          # 32 h-rows per core

# Derived tiling constants
WG = 64                   # matmul row-group (M); window = WG + 96 cols
NG = 128 // WG            # row-groups per 128-row w-block
GW = WG + 2 * MAXD        # gram window width per group
NQ = W // 128             # 4 w-blocks of 128 rows
RPAD_L = MAXD             # left zero pad of r (no right pad: edge windows
                          # are clamped and the host masks the zero band)
WP = W + MAXD             # 560 padded r width
WLR = W + WP              # 1072: combined (l | r_pad) row width
RW = NQ * GW              # per-h-row store width (4 q-windows)
HPS = 8                   # h-rows packed per store DMA descriptor row
SROW = HPS * RW + 1       # skewed DRAM row pitch (8 h-rows + 1)
QDS = 128 * SROW          # per-(b,oct) DRAM region, elems
# PSUM q-window offsets (elems) — each window must stay inside one 2 KiB
# bank (512 f32); for WG=64 (GW=160) q2/q3 move up to the second bank.
if WG == 32:
    PSW = NQ * GW         # 512: one bank
    QOFF = [GW * q for q in range(NQ)]
else:
    PSW = 1024            # two banks
    QOFF = [0, 160, 512, 672]

# module-level result stash (test.py reads these)
LAST_RESULTS = None
_NC_CACHE = {}


def _build_nc(b_n=B, hs=HS):
    """Build the per-core Bass program. All cores run the same program."""
    nc = bass.Bass()
    npairs = hs // 2          # 16 h-pairs per core
    nocts = hs // HPS         # 4 octs (8 h-rows each)
    # lr: [b, hh, c, pair, WLR] — l and r_pad concatenated on W; one load
    # DMA per (b, 8-pair half) covers all 128 partitions (both hh) with
    # 8*WLR-elem (17920 B) descriptors.
    lr_in = nc.dram_tensor("lr", [b_n, 2, C, npairs, WLR], F16,
                           kind="ExternalInput")
    o_out = nc.dram_tensor("o", [b_n, nocts, QDS], F16,
                           kind="ExternalOutput")

    lr_hh = C * npairs * WLR
    lr_c = npairs * WLR
    lr_b = 2 * lr_hh

    with tile.TileContext(nc) as tc:
        with (
            tc.tile_pool(name="lrpool", bufs=4) as lrp,
            tc.tile_pool(name="gpool", bufs=4) as gp,
            tc.tile_pool(name="ppool", bufs=(8 if WG == 32 else 4),
                         space="PSUM") as pp,
        ):
            for b in range(b_n):
                for half in range(2):           # 8 h-pairs per load
                    t0 = half * 8
                    lr_t = lrp.tile([128, 8 * WLR], F16, name="lr_t")
                    lr_src = AP(
                        lr_in,
                        b * lr_b + t0 * WLR,
                        [(lr_hh, 2), (lr_c, C), (1, 8 * WLR)],
                    )
                    # All loads issue from sync: its FIFO holds only loads,
                    # so prefetch is never head-of-line blocked by a store
                    # waiting on evictions (stores live on scalar's FIFO).
                    nc.sync.dma_start(out=lr_t[:, :], in_=lr_src)
                    for st in range(2):         # 2 octs per load
                        g_t = gp.tile([128, HPS * RW], F16, name="g_t")
                        for p4 in range(4):     # h-pair within oct
                            lp = st * 4 + p4    # pair idx within tile
                            ps0 = pp.tile([128, PSW], F32, name="ps0",
                                          tag="ps")
                            ps1 = pp.tile([128, PSW], F32, name="ps1",
                                          tag="ps")
                            pss = (ps0, ps1)
                            for q in range(NQ):
                                for hh in range(2):
                                    for g in range(NG):
                                        c0 = lp * WLR + 128 * q + WG * g
                                        # clamp the window at r_pad's end;
                                        # the host zeros the cut band
                                        nmm = min(GW, WP - (128 * q + WG * g))
                                        lhsT = lr_t[
                                            64 * hh:64 * hh + 64,
                                            c0:c0 + WG,
                                        ]
                                        rhs = lr_t[
                                            64 * hh:64 * hh + 64,
                                            W + c0:W + c0 + nmm,
                                        ]
                                        nc.tensor.matmul(
                                            pss[hh][WG * g:WG * g + WG,
                                                    QOFF[q]:QOFF[q] + nmm],
                                            lhsT, rhs,
                                            start=True, stop=True,
                                            tile_position=(64 * hh, WG * g),
                                        )
                            for hh in range(2):
                                hq = 2 * p4 + hh
                                dst = g_t[:, hq * RW:hq * RW + RW]
                                eng_copy = (
                                    nc.vector.tensor_copy if hh == 0
                                    else nc.scalar.copy
                                )
                                if WG == 32:
                                    eng_copy(dst, pss[hh][:, :])
                                else:
                                    # q0/q1 in bank 0, q2/q3 in bank 1
                                    eng_copy(
                                        g_t[:, hq * RW:hq * RW + 2 * GW],
                                        pss[hh][:, 0:2 * GW],
                                    )
                                    eng_copy(
                                        g_t[:, hq * RW + 2 * GW:
                                            hq * RW + RW],
                                        pss[hh][:, 512:512 + 2 * GW],
                                    )
                        # Skew store: one DMA per oct (8 h-rows); DRAM row
                        # pitch SROW = 8*RW+1 shifts row i by +i within its
                        # WG-row group; groups tile the region exactly.
                        d_ap = AP(
                            o_out,
                            (b * nocts + half * 2 + st) * QDS,
                            [(WG * SROW, NG), (SROW, WG), (1, HPS * RW)],
                        )
                        nc.scalar.dma_start(out=d_ap, in_=g_t[:, :])
    _split_multi_waits(nc)
    return nc


def _split_multi_waits(nc):
    """The 64-byte TPB instruction encoding holds a single semaphore wait;
    walrus codegen rejects instructions whose sync_info carries more. Hoist
    all but one wait onto standalone InstEventSemaphore instructions placed
    immediately before, on the same engine (FIFO order preserves semantics).
    """
    for bb in nc.main_func.blocks:
        new_list = []
        changed = False
        for ins in bb.instructions:
            si = ins.sync_info
            if si is not None and len(si.on_wait) > 1:
                for w in list(si.on_wait)[:-1]:
                    ev = mybir.InstEventSemaphore(
                        name=nc.get_next_instruction_name(),
                        engine=ins.engine,
                        ins=[],
                        outs=[],
                        sync_info=mybir.SyncInfo(on_wait=[w], on_update=[]),
                    )
                    new_list.append(ev)
                ins.sync_info = mybir.SyncInfo(
                    on_wait=[list(si.on_wait)[-1]], on_update=list(si.on_update)
                )
                changed = True
            new_list.append(ins)
        if changed:
            bb.instructions = new_list


def _get_nc(key=(B, HS)):
    if key not in _NC_CACHE:
        _NC_CACHE[key] = _build_nc(*key)
    return _NC_CACHE[key]


def _host_prep(l_fmap, r_fmap):
    """Build lr[b, hh, c, pair, WLR] f16 with l scaled by 1/C and r padded.
    pair runs over all H//2 rows; per-core slices are taken afterwards."""
    l = np.asarray(l_fmap, dtype=np.float32) * np.float32(1.0 / C)
    r = np.asarray(r_fmap, dtype=np.float32)
    l16 = l.astype(np.float16).reshape(B, C, H // 2, 2, W)
    r16 = r.astype(np.float16).reshape(B, C, H // 2, 2, W)
    big = np.zeros((B, 2, C, H // 2, WLR), dtype=np.float16)
    big[..., :W] = l16.transpose(0, 3, 1, 2, 4)
    big[..., W + RPAD_L:W + RPAD_L + W] = r16.transpose(0, 3, 1, 2, 4)
    return big


# (plane, w) positions where w - (plane - MAXD) >= W: r is out of range so
# the reference output is zero there. The device stores garbage for these
# (clamped edge windows); zero them on the host.
_ZMASK = None


def _zero_mask():
    global _ZMASK
    if _ZMASK is None:
        p = np.arange(D)[:, None]
        w = np.arange(W)[None, :]
        _ZMASK = np.nonzero(w - (p - MAXD) >= W)
    return _ZMASK


def _install_ntff_hook_shim(so_path="/opt/axon/libaxon_pjrt.so"):
    """Provide antenv.axon_hooks.get_axon_ntff_profile_hook via ctypes when
    the image's antenv lacks it (mirrors trn_agent_boot's slim hook)."""
    import types
    import ctypes
    import contextlib

    try:
        from antenv.axon_hooks import get_axon_ntff_profile_hook  # noqa: F401
        return
    except ImportError:
        pass

    lib = ctypes.CDLL(so_path)
    if not hasattr(lib, "axon_start_nrt_profile"):
        return
    lib.axon_start_nrt_profile.argtypes = [
        ctypes.POINTER(ctypes.c_int64), ctypes.c_size_t,
    ]
    lib.axon_start_nrt_profile.restype = ctypes.c_int64
    lib.axon_stop_nrt_profile.argtypes = [ctypes.c_char_p]
    lib.axon_stop_nrt_profile.restype = ctypes.c_int64

    @contextlib.contextmanager
    def _hook(output_dir, device_ids):
        import jax
        jax.devices()
        if device_ids:
            ids = (ctypes.c_int64 * len(device_ids))(*device_ids)
            rc = lib.axon_start_nrt_profile(ids, len(device_ids))
        else:
            rc = lib.axon_start_nrt_profile(None, 0)
        if rc != 0:
            raise RuntimeError(f"axon_start_nrt_profile rc={rc}")
        try:
            yield
        finally:
            n = lib.axon_stop_nrt_profile(str(output_dir).encode())
            print(f"ntff profile: {n} file(s) written to {output_dir}",
                  file=sys.stderr)

    import antenv
    mod = types.ModuleType("antenv.axon_hooks")
    mod.get_axon_ntff_profile_hook = lambda: _hook
    mod.set_axon_ntff_profile_hook = lambda h: None
    sys.modules["antenv.axon_hooks"] = mod
    antenv.axon_hooks = mod


def kernel(l_fmap, r_fmap, max_disp):
    global LAST_RESULTS
    assert int(max_disp) == MAXD
    big = _host_prep(l_fmap, r_fmap)   # [B, 2, C, H//2, WLR]

    nc = _get_nc()
    npairs = HS // 2
    in_maps = []
    for k in range(NCORES):
        sl = slice(k * npairs, (k + 1) * npairs)
        in_maps.append({
            "lr": np.ascontiguousarray(big[:, :, :, sl, :]),
        })

    import os
    trace = bool(int(os.environ.get("CV_TRACE", "0")))
    if trace:
        _install_ntff_hook_shim()
    res = run_bass_kernel_spmd(nc, in_maps, list(range(NCORES)), trace=trace)
    LAST_RESULTS = res

    nocts = HS // HPS
    out = np.empty((B, D, H, W), dtype=np.float32)
    for k in range(NCORES):
        o = np.asarray(res.results[k]["o"]).reshape(-1)  # [B*nocts*QDS] f16
        # view axes: (b, oct, g, hq, q, i', delta), delta = k' - i' in [0, D)
        v = np.lib.stride_tricks.as_strided(
            o,
            shape=(B, nocts, NG, HPS, NQ, WG, D),
            strides=tuple(np.array([
                nocts * QDS, QDS, WG * SROW, RW, GW, SROW + 1, 1,
            ]) * o.itemsize),
        )
        # plane index = 96 - delta; h = 8*oct + hq; w = 128*q + WG*g + i'
        t = np.flip(v, axis=6).transpose(0, 6, 1, 3, 4, 2, 5)
        out[:, :, k * HS:(k + 1) * HS, :] = (
            t.reshape(B, D, HS, W).astype(np.float32)
        )
    pz, wz = _zero_mask()
    out[:, pz, :, wz] = 0.0
    return out


# revision 26
# speedup vs baseline: 1.1324x; 1.1324x over previous
"""CostVolume2D Trainium2 kernel (v2: skewed-PSUM compact store).

out[b, d, h, w] = mean_c l[b,c,h,w] * r[b,c,h, w - (d - maxd)]  (zero padded)

Strategy (8 NeuronCores, shard H — no halo since shifts only touch W):
  * Per (b, h): disparity planes are diagonals of banded gram matrices
    G[w, w'] = sum_c l[c, w] r[c, w'] with |w - w'| <= 48.
  * v2 change vs v1: instead of computing full [128 x 224] gram blocks
    and storing all 224 columns per row (2.31x write amplification),
    each 128-row block is split into 128/WG row-groups; the matmul for
    group g streams only the needed column window [WG*g, WG*g+WG+96)
    and writes it to a PSUM window shared across groups. The diagonal
    band is then pre-aligned per group in PSUM, the eviction copies the
    full 128-partition tile at full lane utilization, and the store DMA
    absorbs the residual per-row skew (+1/row within a group) with a
    3-dim flat-stride access pattern. Write amplification drops to
    (WG+96)/97 and the quadrant matmuls (K=64, M=WG) run concurrently
    on disjoint PE array tiles (tile_position auto-derived from the
    lhsT/out base partitions).
  * 4 h-rows are packed per store DMA so descriptors are 8*(WG+96)
    bytes; loads use a host-side relayout so each load descriptor is
    4 h-pairs x 1120 elems (8960 B) contiguous.
  * Host pre-divides l by C (exact, power of two) and pre-pads r along
    W so no on-device scaling / memset / edge handling.
"""

import sys

try:
    import concourse  # noqa: F401
except ImportError:
    sys.path.insert(0, "/opt/trn_rl_repo")

import numpy as np

from concourse import bass, mybir
from concourse import tile
from concourse.ap import AP
from concourse.bass_utils import run_bass_kernel_spmd

F32 = mybir.dt.float32
F16 = mybir.dt.float16

# Problem dims (hardcoded per spec)
B, C, H, W = 4, 64, 256, 512
MAXD = 48
D = 2 * MAXD + 1          # 97 disparity planes
NCORES = 8
HS = H // NCORES          # 32 h-rows per core

# Derived tiling constants
WG = 64                   # matmul row-group (M); window = WG + 96 cols
NG = 128 // WG            # row-groups per 128-row w-block
GW = WG + 2 * MAXD        # gram window width per group
NQ = W // 128             # 4 w-blocks of 128 rows
RPAD_L = MAXD             # left zero pad of r (no right pad: edge windows
                          # are clamped and the host masks the zero band)
WP = W + MAXD             # 560 padded r width
WLR = W + WP              # 1072: combined (l | r_pad) row width
RW = NQ * GW              # per-h-row store width (4 q-windows)
HPS = 8                   # h-rows packed per store DMA descriptor row
SROW = HPS * RW + 1       # skewed DRAM row pitch (8 h-rows + 1)
QDS = 128 * SROW          # per-(b,oct) DRAM region, elems
# PSUM q-window offsets — each window must stay inside one 2 KiB bank
# (512 f32). WG=32: all four windows fit one bank. WG=64 (GW=160): PSUM
# tiles are [128, 2, 512] (two banks); window q lives in bank q//2 at
# offset 160*(q%2).
if WG == 32:
    PSW = NQ * GW         # 512: one bank
    QOFF = [GW * q for q in range(NQ)]
else:
    PSW = 1024            # two banks
    QOFF = [(q // 2, GW * (q % 2)) for q in range(NQ)]

# module-level result stash (test.py reads these)
LAST_RESULTS = None
_NC_CACHE = {}


def _build_nc(b_n=B, hs=HS):
    """Build the per-core Bass program. All cores run the same program."""
    nc = bass.Bass()
    npairs = hs // 2          # 16 h-pairs per core
    nocts = hs // HPS         # 4 octs (8 h-rows each)
    # lr: [b, hh, c, pair, WLR] — l and r_pad concatenated on W; one load
    # DMA per (b, 8-pair half) covers all 128 partitions (both hh) with
    # 8*WLR-elem (17920 B) descriptors.
    lr_in = nc.dram_tensor("lr", [b_n, 2, C, npairs, WLR], F16,
                           kind="ExternalInput")
    o_out = nc.dram_tensor("o", [b_n, nocts, QDS], F16,
                           kind="ExternalOutput")

    lr_hh = C * npairs * WLR
    lr_c = npairs * WLR
    lr_b = 2 * lr_hh

    with tile.TileContext(nc) as tc:
        with (
            tc.tile_pool(name="lrpool", bufs=4) as lrp,
            tc.tile_pool(name="gpool", bufs=4) as gp,
            tc.tile_pool(name="ppool", bufs=(8 if WG == 32 else 4),
                         space="PSUM") as pp,
        ):
            # HAM warm-up: ~40 dummy matmuls during the first load's DMA
            # (PE is otherwise idle) so the clock gate reaches K=8/8 before
            # the first real matmul.
            wz = gp.tile([64, 128], F16, name="wz", tag="warm", bufs=1)
            nc.vector.memset(wz[:, :], 0)
            warm_shape = [128, PSW] if WG == 32 else [128, 2, 512]
            warm_ps = pp.tile(warm_shape, F32, name="warm_ps", tag="ps")
            warm_out = (warm_ps[:, 0:128] if WG == 32
                        else warm_ps[:, 0, 0:128])
            for _ in range(40):
                nc.tensor.matmul(warm_out, wz[:, :], wz[:, :],
                                 start=True, stop=True)
            for b in range(b_n):
                for half in range(2):           # 8 h-pairs per load
                    t0 = half * 8
                    lr_t = lrp.tile([128, 8 * WLR], F16, name="lr_t")
                    lr_src = AP(
                        lr_in,
                        b * lr_b + t0 * WLR,
                        [(lr_hh, 2), (lr_c, C), (1, 8 * WLR)],
                    )
                    # All loads issue from sync: its FIFO holds only loads,
                    # so prefetch is never head-of-line blocked by a store
                    # waiting on evictions (stores live on scalar's FIFO).
                    nc.sync.dma_start(out=lr_t[:, :], in_=lr_src)
                    for st in range(2):         # 2 octs per load
                        g_t = gp.tile([128, HPS * RW], F16, name="g_t")
                        for p4 in range(4):     # h-pair within oct
                            lp = st * 4 + p4    # pair idx within tile
                            ps_shape = ([128, PSW] if WG == 32
                                        else [128, 2, 512])
                            ps0 = pp.tile(ps_shape, F32, name="ps0",
                                          tag="ps")
                            ps1 = pp.tile(ps_shape, F32, name="ps1",
                                          tag="ps")
                            pss = (ps0, ps1)
                            for q in range(NQ):
                                for hh in range(2):
                                    for g in range(NG):
                                        c0 = lp * WLR + 128 * q + WG * g
                                        # clamp the window at r_pad's end;
                                        # the host zeros the cut band
                                        nmm = min(GW, WP - (128 * q + WG * g))
                                        lhsT = lr_t[
                                            64 * hh:64 * hh + 64,
                                            c0:c0 + WG,
                                        ]
                                        rhs = lr_t[
                                            64 * hh:64 * hh + 64,
                                            W + c0:W + c0 + nmm,
                                        ]
                                        if WG == 32:
                                            mm_out = pss[hh][
                                                WG * g:WG * g + WG,
                                                QOFF[q]:QOFF[q] + nmm]
                                        else:
                                            bk, off = QOFF[q]
                                            mm_out = pss[hh][
                                                WG * g:WG * g + WG,
                                                bk, off:off + nmm]
                                        nc.tensor.matmul(
                                            mm_out, lhsT, rhs,
                                            start=True, stop=True,
                                            tile_position=(64 * hh, WG * g),
                                        )
                            for hh in range(2):
                                hq = 2 * p4 + hh
                                dst = g_t[:, hq * RW:hq * RW + RW]
                                eng_copy = (
                                    nc.vector.tensor_copy if hh == 0
                                    else nc.scalar.copy
                                )
                                if WG == 32:
                                    eng_copy(dst, pss[hh][:, :])
                                else:
                                    # one strided copy: both banks' 320-elem
                                    # windows -> contiguous 640 in g_t
                                    eng_copy(
                                        dst.rearrange(
                                            "p (a b) -> p a b", a=2),
                                        pss[hh][:, :, 0:2 * GW],
                                    )
                        # Skew store: one DMA per oct (8 h-rows); DRAM row
                        # pitch SROW = 8*RW+1 shifts row i by +i within its
                        # WG-row group; groups tile the region exactly.
                        d_ap = AP(
                            o_out,
                            (b * nocts + half * 2 + st) * QDS,
                            [(WG * SROW, NG), (SROW, WG), (1, HPS * RW)],
                        )
                        nc.scalar.dma_start(out=d_ap, in_=g_t[:, :])
    _split_multi_waits(nc)
    return nc


def _split_multi_waits(nc):
    """The 64-byte TPB instruction encoding holds a single semaphore wait;
    walrus codegen rejects instructions whose sync_info carries more. Hoist
    all but one wait onto standalone InstEventSemaphore instructions placed
    immediately before, on the same engine (FIFO order preserves semantics).
    """
    for bb in nc.main_func.blocks:
        new_list = []
        changed = False
        for ins in bb.instructions:
            si = ins.sync_info
            if si is not None and len(si.on_wait) > 1:
                for w in list(si.on_wait)[:-1]:
                    ev = mybir.InstEventSemaphore(
                        name=nc.get_next_instruction_name(),
                        engine=ins.engine,
                        ins=[],
                        outs=[],
                        sync_info=mybir.SyncInfo(on_wait=[w], on_update=[]),
                    )
                    new_list.append(ev)
                ins.sync_info = mybir.SyncInfo(
                    on_wait=[list(si.on_wait)[-1]], on_update=list(si.on_update)
                )
                changed = True
            new_list.append(ins)
        if changed:
            bb.instructions = new_list


def _get_nc(key=(B, HS)):
    if key not in _NC_CACHE:
        _NC_CACHE[key] = _build_nc(*key)
    return _NC_CACHE[key]


def _host_prep(l_fmap, r_fmap):
    """Build lr[b, hh, c, pair, WLR] f16 with l scaled by 1/C and r padded.
    pair runs over all H//2 rows; per-core slices are taken afterwards."""
    l = np.asarray(l_fmap, dtype=np.float32) * np.float32(1.0 / C)
    r = np.asarray(r_fmap, dtype=np.float32)
    l16 = l.astype(np.float16).reshape(B, C, H // 2, 2, W)
    r16 = r.astype(np.float16).reshape(B, C, H // 2, 2, W)
    big = np.zeros((B, 2, C, H // 2, WLR), dtype=np.float16)
    big[..., :W] = l16.transpose(0, 3, 1, 2, 4)
    big[..., W + RPAD_L:W + RPAD_L + W] = r16.transpose(0, 3, 1, 2, 4)
    return big


# (plane, w) positions where w - (plane - MAXD) >= W: r is out of range so
# the reference output is zero there. The device stores garbage for these
# (clamped edge windows); zero them on the host.
_ZMASK = None


def _zero_mask():
    global _ZMASK
    if _ZMASK is None:
        p = np.arange(D)[:, None]
        w = np.arange(W)[None, :]
        _ZMASK = np.nonzero(w - (p - MAXD) >= W)
    return _ZMASK


def _install_ntff_hook_shim(so_path="/opt/axon/libaxon_pjrt.so"):
    """Provide antenv.axon_hooks.get_axon_ntff_profile_hook via ctypes when
    the image's antenv lacks it (mirrors trn_agent_boot's slim hook)."""
    import types
    import ctypes
    import contextlib

    try:
        from antenv.axon_hooks import get_axon_ntff_profile_hook  # noqa: F401
        return
    except ImportError:
        pass

    lib = ctypes.CDLL(so_path)
    if not hasattr(lib, "axon_start_nrt_profile"):
        return
    lib.axon_start_nrt_profile.argtypes = [
        ctypes.POINTER(ctypes.c_int64), ctypes.c_size_t,
    ]
    lib.axon_start_nrt_profile.restype = ctypes.c_int64
    lib.axon_stop_nrt_profile.argtypes = [ctypes.c_char_p]
    lib.axon_stop_nrt_profile.restype = ctypes.c_int64

    @contextlib.contextmanager
    def _hook(output_dir, device_ids):
        import jax
        jax.devices()
        if device_ids:
            ids = (ctypes.c_int64 * len(device_ids))(*device_ids)
            rc = lib.axon_start_nrt_profile(ids, len(device_ids))
        else:
            rc = lib.axon_start_nrt_profile(None, 0)
        if rc != 0:
            raise RuntimeError(f"axon_start_nrt_profile rc={rc}")
        try:
            yield
        finally:
            n = lib.axon_stop_nrt_profile(str(output_dir).encode())
            print(f"ntff profile: {n} file(s) written to {output_dir}",
                  file=sys.stderr)

    import antenv
    mod = types.ModuleType("antenv.axon_hooks")
    mod.get_axon_ntff_profile_hook = lambda: _hook
    mod.set_axon_ntff_profile_hook = lambda h: None
    sys.modules["antenv.axon_hooks"] = mod
    antenv.axon_hooks = mod


def kernel(l_fmap, r_fmap, max_disp):
    global LAST_RESULTS
    assert int(max_disp) == MAXD
    big = _host_prep(l_fmap, r_fmap)   # [B, 2, C, H//2, WLR]

    nc = _get_nc()
    npairs = HS // 2
    in_maps = []
    for k in range(NCORES):
        sl = slice(k * npairs, (k + 1) * npairs)
        in_maps.append({
            "lr": np.ascontiguousarray(big[:, :, :, sl, :]),
        })

    import os
    trace = bool(int(os.environ.get("CV_TRACE", "0")))
    if trace:
        _install_ntff_hook_shim()
    res = run_bass_kernel_spmd(nc, in_maps, list(range(NCORES)), trace=trace)
    LAST_RESULTS = res

    nocts = HS // HPS
    out = np.empty((B, D, H, W), dtype=np.float32)
    for k in range(NCORES):
        o = np.asarray(res.results[k]["o"]).reshape(-1)  # [B*nocts*QDS] f16
        # view axes: (b, oct, g, hq, q, i', delta), delta = k' - i' in [0, D)
        v = np.lib.stride_tricks.as_strided(
            o,
            shape=(B, nocts, NG, HPS, NQ, WG, D),
            strides=tuple(np.array([
                nocts * QDS, QDS, WG * SROW, RW, GW, SROW + 1, 1,
            ]) * o.itemsize),
        )
        # plane index = 96 - delta; h = 8*oct + hq; w = 128*q + WG*g + i'
        t = np.flip(v, axis=6).transpose(0, 6, 1, 3, 4, 2, 5)
        out[:, :, k * HS:(k + 1) * HS, :] = (
            t.reshape(B, D, HS, W).astype(np.float32)
        )
    pz, wz = _zero_mask()
    out[:, pz, :, wz] = 0.0
    return out
